# revision 1
# baseline (speedup 1.0000x reference)
"""CQAttention (QANet context-query attention) Trainium2 Bass kernel.

Full-input contract: kernel(C, Q, cmask, qmask, w) -> (B, 4D, LC) f32.
Shards batch B=16 across 8 NeuronCores (2 examples/core), runs one SPMD
Bass/Tile program, gathers results.

Math (per example, d=512, Lc=2048, Lq=512):
  S = Cb@w1 [i] + Qb@w2 [j] + (Cb*w3)@Qb^T          (Lc, Lq)
  S1 = softmax_j(S), S2 = softmax_i(S)
  A = S1@Qb ; Bt = S1@S2^T@Cb
  out = concat([Cb, A, Cb*A, Cb*Bt], feat).T        (4d, Lc)

Kernel structure (all layouts "feature-on-partitions" = input layout of
C/Q = required output layout):
  - softmax shift-invariance drops each softmax's invariant bias term:
      E2  = exp(S + r1)   = exp(C^T_chunks @ (w3*Q + w1))  rows=i, cols=j
      E1T = exp(S^T + c2) = exp((w3*Q)^T_chunks @ C + c2)  rows=j, cols=i
    where c2 = Q^T w2 enters as a per-partition activation bias.
    (max-subtraction skipped: |S + bias| <= ~8 for N(0,1)-scale inputs)
  - partition-dim sums via ones-vector matmuls. The axis-j softmax
    normalization is replicated across partitions via a K=1 ones
    outer-product matmul and folded INTO E1T (in place), so A^T and
    Bt^T come out of their matmuls fully normalized; the axis-i one is
    a per-partition tensor_scalar on T2 = S2raw^T@Cb.
  - A^T = Qb @ E1Tn, Bt^T = T2s^T @ E1Tn; output rows are elementwise
    products with re-loaded fp32 C rows.
  - matmuls in float32r (full PE rate at N=512); f32r operands must be
    produced rounded, so they're written by DVE/ACT into f32r tiles.
  - pools are shared across both examples (tag-level dependencies, no
    pool barriers); emission order software-pipelines the examples:
    example n+1's load/cast/transpose phase is emitted before example
    n's output phase so PE never drains behind the DVE FIFO, and the
    dependent ssum/T2/colsum chains trail their producers by one step.
"""

import numpy as np

import concourse.bass as bass
import concourse.tile as tile
from concourse import bacc, mybir
from concourse.bass_utils import run_bass_kernel_spmd
from concourse.masks import make_identity

B, D, LC, LQ = 16, 512, 2048, 512
NCORES = 8
BL = B // NCORES  # examples per core
KD = D // 128  # 4 d-chunks
KJ = LQ // 128  # 4 j-chunks
NI = LC // 512  # 4 i column-chunks
MI = LC // 128  # 16 i partition-chunks

F32 = mybir.dt.float32
F32R = mybir.dt.float32r
EXP = mybir.ActivationFunctionType.Exp
COPY = mybir.ActivationFunctionType.Copy
MUL = mybir.AluOpType.mult
ADD = mybir.AluOpType.add


class Ctx:
    pass


def _pools(tc, ctx):
    P = Ctx()
    P.const = ctx.enter_context(tc.tile_pool(name="const", bufs=1))
    P.cstage = ctx.enter_context(tc.tile_pool(name="cstage", bufs=2))
    P.qt = ctx.enter_context(tc.tile_pool(name="qt", bufs=1))
    P.big = ctx.enter_context(tc.tile_pool(name="big", bufs=1))
    P.mid = ctx.enter_context(tc.tile_pool(name="mid", bufs=1))
    P.stream = ctx.enter_context(tc.tile_pool(name="stream", bufs=1))
    P.ost = ctx.enter_context(tc.tile_pool(name="ost", bufs=2))
    P.psum = ctx.enter_context(tc.tile_pool(name="psum", space="PSUM", bufs=8))
    return P


def _phase_A(nc, P, K, T, Cd, Qd, Od, b):
    """Loads, rounded/scaled operands, Q transpose, c2 bias columns."""
    psum = P.psum
    Qt = P.qt.tile([128, KD, LQ], F32, tag="qt", name=f"qt{b}")
    for a in range(KD):
        nc.sync.dma_start(
            out=Qt[:, a, :], in_=Qd[b, a * 128 : (a + 1) * 128, :]
        )
    QtR = P.qt.tile([128, KD, LQ], F32R, tag="qtr", name=f"qtr{b}")
    T.QW3 = P.mid.tile([128, KD, LQ], F32R, tag="qw3", name=f"qw3{b}")
    T.Qmod = P.mid.tile([128, KD, LQ], F32R, tag="qmod", name=f"qmod{b}")
    T.CtR = P.mid.tile([128, KD, LC], F32R, tag="cbig", name=f"ctr{b}")
    for k in range(KD):
        # wsb cols: 0-3 w1, 4-7 w2, 8-11 w3
        nc.vector.tensor_copy(QtR[:, k, :], Qt[:, k, :])
        nc.vector.tensor_scalar(
            out=T.QW3[:, k, :], in0=Qt[:, k, :],
            scalar1=K.wsb[:, 8 + k : 9 + k], scalar2=None, op0=MUL,
        )
        nc.vector.tensor_scalar(
            out=T.Qmod[:, k, :], in0=Qt[:, k, :],
            scalar1=K.wsb[:, 8 + k : 9 + k], scalar2=K.wsb[:, k : k + 1],
            op0=MUL, op1=ADD,
        )
    qps = [
        psum.tile([128, D], F32, tag="ps", name=f"qps{b}_{c}") for c in range(KJ)
    ]
    for a in range(KD):
        for c in range(KJ):
            nc.tensor.transpose(
                qps[c][:, a * 128 : (a + 1) * 128],
                Qt[:, a, c * 128 : (c + 1) * 128],
                K.ident,
            )
    for c in range(KJ):
        nc.vector.tensor_copy(T.Qbt[:, c, :], qps[c])
    # c2[j] = Q^T w2, computed as a row then transposed to per-partition
    # columns (fp32r matmuls require a wide moving operand)
    c2row_ps = psum.tile([1, LQ], F32, tag="ps", name=f"c2rp{b}")
    for kd in range(KD):
        nc.tensor.matmul(
            c2row_ps, K.wsbR[:, 4 + kd : 5 + kd], QtR[:, kd, :],
            start=(kd == 0), stop=(kd == KD - 1),
        )
    c2row = P.stream.tile([1, LQ], F32, tag="c2row", name=f"c2r{b}")
    nc.vector.tensor_copy(c2row, c2row_ps)
    c2ps = psum.tile([128, KJ], F32, tag="ps", name=f"c2ps{b}")
    for jm in range(KJ):
        nc.tensor.transpose(
            c2ps[:, jm : jm + 1],
            c2row[:, jm * 128 : (jm + 1) * 128],
            K.ident[:1, :1],
        )
    T.c2col = P.mid.tile([128, KJ], F32, tag="c2col", name=f"c2col{b}")
    nc.vector.tensor_copy(T.c2col, c2ps)
    for k in range(KD):
        cst = P.cstage.tile([128, LC], F32, tag="cstage", name=f"cst{b}_{k}")
        nc.sync.dma_start(out=cst, in_=Cd[b, k * 128 : (k + 1) * 128, :])
        # out rows 0..D-1 are exactly C[b]
        nc.sync.dma_start(out=Od[b, k * 128 : (k + 1) * 128, :], in_=cst)
        nc.vector.tensor_copy(T.CtR[:, k, :], cst)


def _phase_CD(nc, P, K, T, b):
    """Stream E2 row-chunks -> T2 accumulation + ssum2. The ssum/T2
    consumers trail the transpose/E2 producers by one step so PE never
    waits on ACT's exp."""
    psum = P.psum
    t2ps = [
        psum.tile([128, D], F32, tag="ps", name=f"t2ps{b}_{m}") for m in range(KJ)
    ]
    ssps = psum.tile([1, LQ], F32, tag="ps", name=f"ssps{b}")
    T.t2ps, T.ssps = t2ps, ssps
    e2sbs, cbt_sbs = {}, {}

    def consume(ki):
        e2sb = e2sbs.pop(ki)
        nc.tensor.matmul(
            ssps, K.ones_col, e2sb, start=(ki == 0), stop=(ki == MI - 1)
        )
        for mj in range(KJ):
            nc.tensor.matmul(
                t2ps[mj], e2sb[:, mj * 128 : (mj + 1) * 128], cbt_sbs.pop(ki)
                if mj == KJ - 1 else cbt_sbs[ki],
                start=(ki == 0), stop=(ki == MI - 1),
            )

    for ki in range(MI):
        isl = slice(ki * 128, (ki + 1) * 128)
        cbt_ps = psum.tile([128, D], F32R, tag="ps", name=f"cps{b}_{ki}")
        for kd in range(KD):
            nc.tensor.transpose(
                cbt_ps[:, kd * 128 : (kd + 1) * 128], T.CtR[:, kd, isl], K.identR
            )
        cbt_sb = P.stream.tile(
            [128, D], F32R, tag="cbt", bufs=3, name=f"cbt{b}_{ki}"
        )
        nc.vector.tensor_copy(cbt_sb, cbt_ps)
        cbt_sbs[ki] = cbt_sb

        e2ps = psum.tile([128, LQ], F32, tag="ps", name=f"e2ps{b}_{ki}")
        for kd in range(KD):
            nc.tensor.matmul(
                e2ps, T.CtR[:, kd, isl], T.Qmod[:, kd, :],
                start=(kd == 0), stop=(kd == KD - 1),
            )
        e2sb = P.stream.tile([128, LQ], F32R, tag="e2", bufs=3, name=f"e2sb{b}_{ki}")
        nc.scalar.activation(e2sb, e2ps, EXP)
        e2sbs[ki] = e2sb
        if ki > 0:
            consume(ki - 1)
    consume(MI - 1)


def _colsum_block(nc, P, K, T, b, ni):
    """Column sums of E1T -> replicated reciprocal -> normalize E1T in
    place so downstream A/Bt matmuls come out normalized."""
    psum = P.psum
    nsl = slice(ni * 512, (ni + 1) * 512)
    csps = psum.tile([1, 512], F32, tag="ps", name=f"csps{b}_{ni}")
    for kj in range(KJ):
        nc.tensor.matmul(
            csps, K.ones_col, T.E1T[:, kj, nsl],
            start=(kj == 0), stop=(kj == KJ - 1),
        )
    csrow = P.stream.tile([1, 512], F32R, tag="csrow", name=f"cs{b}_{ni}")
    nc.vector.tensor_copy(csrow, csps)
    repps = psum.tile([128, 512], F32, tag="ps", name=f"repps{b}_{ni}")
    nc.tensor.matmul(repps, K.ones_row, csrow, start=True, stop=True)
    nc.vector.reciprocal(T.rec1rep[:, ni, :], repps)
    for kj in range(KJ):
        nc.vector.tensor_mul(
            T.E1T[:, kj, nsl], T.E1T[:, kj, nsl], T.rec1rep[:, ni, :]
        )


def _rec2_block(nc, P, K, T, b):
    rec2row = P.stream.tile([1, LQ], F32, tag="rec2row", name=f"r2r{b}")
    nc.vector.reciprocal(rec2row, T.ssps)
    rc_ps = P.psum.tile([128, KJ], F32, tag="ps", name=f"rcps{b}")
    for jm in range(KJ):
        nc.tensor.transpose(
            rc_ps[:, jm : jm + 1],
            rec2row[:, jm * 128 : (jm + 1) * 128],
            K.ident[:1, :1],
        )
    rec2col = P.stream.tile([128, KJ], F32, tag="rec2col", name=f"r2c{b}")
    nc.vector.tensor_copy(rec2col, rc_ps)
    for mj in range(KJ):
        nc.vector.tensor_scalar(
            out=T.T2s[:, mj, :], in0=T.t2ps[mj],
            scalar1=rec2col[:, mj : mj + 1], scalar2=None, op0=MUL,
        )


def _phase_B(nc, P, K, T, b):
    """E1T = exp((w3*Q)^T_chunks @ C + c2), column-outer, with each
    column's colsum block trailing by one step. The last block is
    deferred into the next emitted phase (returned as a closure)."""
    psum = P.psum
    for ni in range(NI):
        nsl = slice(ni * 512, (ni + 1) * 512)
        for mj in range(KJ):
            e1ps = psum.tile([128, 512], F32, tag="ps", name=f"e1ps{b}_{mj}_{ni}")
            for kd in range(KD):
                nc.tensor.matmul(
                    e1ps,
                    T.QW3[:, kd, mj * 128 : (mj + 1) * 128],
                    T.CtR[:, kd, nsl],
                    start=(kd == 0), stop=(kd == KD - 1),
                )
            nc.scalar.activation(
                T.E1T[:, mj, nsl], e1ps, EXP, bias=T.c2col[:, mj : mj + 1]
            )
        if ni == 0:
            _rec2_block(nc, P, K, T, b)
        else:
            _colsum_block(nc, P, K, T, b, ni - 1)
    return lambda: _colsum_block(nc, P, K, T, b, NI - 1)


def _phase_E(nc, P, K, T, Cd, Od, b, pending=None):
    """A^T, C*A^T, C*Bt^T (rows d, cols i). E1T is pre-normalized, so
    A/Bt matmuls need no further scaling: o2 is an ACT copy, o3/o4 are
    single DVE multiplies with re-loaded fp32 C rows."""
    psum = P.psum
    for md in range(4):
        msl = slice(md * 128, (md + 1) * 128)
        cte = P.cstage.tile([128, LC], F32, tag="cstage", name=f"cte{b}_{md}")
        nc.sync.dma_start(out=cte, in_=Cd[b, md * 128 : (md + 1) * 128, :])
        for h in range(2):
            hsl = slice(h * 1024, (h + 1) * 1024)
            o2 = P.ost.tile([128, 1024], F32, tag="o2", name=f"o2_{b}_{md}_{h}")
            o3 = P.ost.tile([128, 1024], F32, tag="o3", name=f"o3_{b}_{md}_{h}")
            for ni in (2 * h, 2 * h + 1):
                nsl = slice(ni * 512, (ni + 1) * 512)
                osl = slice((ni - 2 * h) * 512, (ni - 2 * h + 1) * 512)
                aps = psum.tile([128, 512], F32, tag="ps", name=f"aps{b}_{md}_{ni}")
                for kj in range(KJ):
                    nc.tensor.matmul(
                        aps, T.Qbt[:, kj, msl], T.E1T[:, kj, nsl],
                        start=(kj == 0), stop=(kj == KJ - 1),
                    )
                if pending is not None:
                    pending()
                    pending = None
                nc.scalar.activation(o2[:, osl], aps, COPY)
                nc.vector.tensor_mul(o3[:, osl], o2[:, osl], cte[:, nsl])
            nc.sync.dma_start(
                out=Od[b, D + md * 128 : D + (md + 1) * 128, hsl], in_=o2
            )
            nc.sync.dma_start(
                out=Od[b, 2 * D + md * 128 : 2 * D + (md + 1) * 128, hsl], in_=o3
            )
        for h in range(2):
            hsl = slice(h * 1024, (h + 1) * 1024)
            o4 = P.ost.tile([128, 1024], F32, tag="o4", name=f"o4_{b}_{md}_{h}")
            for ni in (2 * h, 2 * h + 1):
                nsl = slice(ni * 512, (ni + 1) * 512)
                osl = slice((ni - 2 * h) * 512, (ni - 2 * h + 1) * 512)
                bps = psum.tile([128, 512], F32, tag="ps", name=f"bps{b}_{md}_{ni}")
                for kj in range(KJ):
                    nc.tensor.matmul(
                        bps, T.T2s[:, kj, msl], T.E1T[:, kj, nsl],
                        start=(kj == 0), stop=(kj == KJ - 1),
                    )
                nc.vector.tensor_mul(o4[:, osl], bps, cte[:, nsl])
            nc.sync.dma_start(
                out=Od[b, 3 * D + md * 128 : 3 * D + (md + 1) * 128, hsl], in_=o4
            )


def build(bl=BL, num_devices=NCORES, enable_asserts=False):
    from contextlib import ExitStack

    nc = bacc.Bacc(
        "TRN2",
        target_bir_lowering=False,
        debug=False,
        enable_asserts=enable_asserts,
        num_devices=num_devices,
    )
    Cd = nc.dram_tensor("C", (bl, D, LC), F32, kind="ExternalInput").ap()
    Qd = nc.dram_tensor("Q", (bl, D, LQ), F32, kind="ExternalInput").ap()
    wd = nc.dram_tensor("w", (3 * D,), F32, kind="ExternalInput").ap()
    Od = nc.dram_tensor("out", (bl, 4 * D, LC), F32, kind="ExternalOutput").ap()

    with tile.TileContext(nc) as tc, ExitStack() as ctx:
        P = _pools(tc, ctx)
        K = Ctx()
        K.ident = P.const.tile([128, 128], F32, name="ident")
        make_identity(nc, K.ident)
        K.identR = P.const.tile([128, 128], F32R, name="identR")
        nc.vector.tensor_copy(K.identR, K.ident)
        ones_col_f = P.const.tile([128, 1], F32, name="ocf")
        nc.vector.memset(ones_col_f, 1.0)
        K.ones_col = P.const.tile([128, 1], F32R, name="oc")
        nc.vector.tensor_copy(K.ones_col, ones_col_f)
        ones_row_f = P.const.tile([1, 128], F32, name="orf")
        nc.vector.memset(ones_row_f, 1.0)
        K.ones_row = P.const.tile([1, 128], F32R, name="orr")
        nc.vector.tensor_copy(K.ones_row, ones_row_f)
        K.wsb = P.const.tile([128, 12], F32, name="wsb")
        nc.sync.dma_start(out=K.wsb, in_=wd.rearrange("(c p) -> p c", p=128))
        K.wsbR = P.const.tile([128, 12], F32R, name="wsbR")
        nc.vector.tensor_copy(K.wsbR, K.wsb)

        tiles, pend = {}, {}
        for b in range(bl):
            T = tiles[b] = Ctx()
            T.E1T = P.big.tile([128, KJ, LC], F32R, tag="e1t", name=f"e1t{b}")
            T.Qbt = P.big.tile([128, KJ, D], F32R, tag="qbt", bufs=2, name=f"qbt{b}")
            T.T2s = P.big.tile([128, KJ, D], F32R, tag="t2s", name=f"t2s{b}")
            T.rec1rep = P.big.tile(
                [128, NI, 512], F32, tag="rec1", name=f"rc1{b}"
            )
            _phase_A(nc, P, K, T, Cd, Qd, Od, b)
            if b > 0:
                _phase_E(nc, P, K, tiles[b - 1], Cd, Od, b - 1, pend[b - 1])
            _phase_CD(nc, P, K, T, b)
            pend[b] = _phase_B(nc, P, K, T, b)
        _phase_E(nc, P, K, tiles[bl - 1], Cd, Od, bl - 1, pend[bl - 1])
    nc.compile()
    return nc


_NC = None


def kernel(C, Q, cmask, qmask, w):
    global _NC
    C = np.ascontiguousarray(np.asarray(C, dtype=np.float32))
    Q = np.ascontiguousarray(np.asarray(Q, dtype=np.float32))
    w = np.ascontiguousarray(np.asarray(w, dtype=np.float32))
    # masks are all-ones per the problem spec; softmax masking is a no-op
    if _NC is None:
        _NC = build()
    in_maps = [
        {
            "C": np.ascontiguousarray(C[i * BL : (i + 1) * BL]),
            "Q": np.ascontiguousarray(Q[i * BL : (i + 1) * BL]),
            "w": w,
        }
        for i in range(NCORES)
    ]
    res = run_bass_kernel_spmd(_NC, in_maps, core_ids=list(range(NCORES)))
    return np.concatenate([res.results[i]["out"] for i in range(NCORES)], axis=0)



# revision 39
# speedup vs baseline: 1.1992x; 1.1992x over previous
"""CQAttention (QANet context-query attention) Trainium2 Bass kernel.

Full-input contract: kernel(C, Q, cmask, qmask, w) -> (B, 4D, LC) f32.
Shards batch B=16 across 8 NeuronCores (2 examples/core), runs one SPMD
Bass/Tile program, gathers results.

Math (per example, d=512, Lc=2048, Lq=512):
  S = Cb@w1 [i] + Qb@w2 [j] + (Cb*w3)@Qb^T          (Lc, Lq)
  S1 = softmax_j(S), S2 = softmax_i(S)
  A = S1@Qb ; Bt = S1@S2^T@Cb
  out = concat([Cb, A, Cb*A, Cb*Bt], feat).T        (4d, Lc)

Design (vs the f32r baseline at 287us; this version measures ~238us):
  - all matmul operands are bf16 (tolerance 2e-2; bf16 chain lands ~3e-3)
  - S is computed ONCE (rows i): a K=1 rank-1 matmul (ones x c2row) plus
    C^T_chunks @ (Q*w3 + w1) accumulate S = M + c1 + c2 in PSUM. One ACT
    exp with fused accum_out yields e2c = exp(S) AND the row-softmax
    normalizer R1 in a single instruction. e2cn = e2c/R1 = S1 rows (DVE
    per-partition scale).
  - S1^T (j-on-partitions, needed for the A/Bt matmuls) comes from the
    XBAR DMA-transpose ucode (14ns per 16x128 tile) on the otherwise
    empty SP ring instead of a second 1-GFLOP PE matmul. C^T/Q^T chunks
    use PE transposes (the XBAR config cost, ~2.4us serial per
    transpose on the SP engine+sequencer, only pays off for the 16
    latency-tolerant S1^T transposes per example).
  - S2 path: T2' = e2c^T-chunks @ cbt accumulated in PSUM; R2' via ones
    matmul; exp(c2) cancels in T2'/R2'. T2s = S2^T@Cb in bf16.
  - A-path outputs (o2=A^T, o3=C*A^T) are produced inline per 512-col
    i-block as soon as its E1T columns land, spreading output DMA across
    the chunk loop instead of a tail burst. Bt-path (o4) streams after
    the (unavoidable) all-i barrier.
  - phase E multiplies against the bf16 C already in SBUF (no C reload;
    saves 8.4MB of HBM traffic per core).
  - DMA ring separation: inputs + S1^T transposes on the SP HWDGE ring,
    o1/o2/o4-odd stores on the scalar HWDGE ring, o3/o4-even on the
    gpsimd SWDGE ring, so latency-critical transposes never queue
    behind bulk output writes and no single sequencer sees config
    bursts between compute dispatches.
  - T2/ssum consumers trail their chunk's exp by two chunks and stream
    tiles are deep (4-8 bufs) so PE never waits on the ACT exp / XBAR
    transpose chain; PSUM rotates within 8 banks (4 persistent T2
    accumulators + ssum + e2ps/cbt/aps rotation).
"""

import numpy as np

import concourse.bass as bass
import concourse.tile as tile
from concourse import bacc, mybir
from concourse.bass_utils import run_bass_kernel_spmd
from concourse.masks import make_identity

B, D, LC, LQ = 16, 512, 2048, 512
NCORES = 8
BL = B // NCORES  # examples per core
KD = D // 128  # 4 d-chunks
KJ = LQ // 128  # 4 j-chunks
NI = LC // 512  # 4 i column-blocks (phase E granularity)
MI = LC // 128  # 16 i partition-chunks

F32 = mybir.dt.float32
BF = mybir.dt.bfloat16
EXP = mybir.ActivationFunctionType.Exp
COPY = mybir.ActivationFunctionType.Copy
MUL = mybir.AluOpType.mult
ADD = mybir.AluOpType.add


class Ctx:
    pass


def _pools(tc, ctx):
    P = Ctx()
    P.const = ctx.enter_context(tc.tile_pool(name="const", bufs=1))
    P.cstage = ctx.enter_context(tc.tile_pool(name="cstage", bufs=2))
    P.qt = ctx.enter_context(tc.tile_pool(name="qt", bufs=1))
    P.big = ctx.enter_context(tc.tile_pool(name="big", bufs=1))
    P.mid = ctx.enter_context(tc.tile_pool(name="mid", bufs=1))
    P.stream = ctx.enter_context(tc.tile_pool(name="stream", bufs=1))
    P.ost = ctx.enter_context(tc.tile_pool(name="ost", bufs=3))
    P.psum = ctx.enter_context(tc.tile_pool(name="psum", space="PSUM", bufs=8))
    return P


def _loads(nc, P, K, T, Cd, Qd, b, first=False):
    """Input DMAs. The first example's C loads ride the (empty) SP ring
    so their HWDGE generation runs in parallel with the gpsimd SWDGE
    generation of the Q loads; prefetched examples keep everything on
    the SWDGE ring, away from the SP transpose chain."""
    eng = nc.sync if first else nc.gpsimd
    T.cst = []
    for k in range(KD):
        cst = P.cstage.tile([128, LC], F32, tag="cstage", bufs=2, name=f"cst{b}_{k}")
        eng.dma_start(out=cst, in_=Cd[b, k * 128 : (k + 1) * 128, :])
        T.cst.append(cst)
    T.Qt = P.qt.tile([128, KD, LQ], F32, tag="qt", bufs=2, name=f"qt{b}")
    for a in range(KD):
        eng.dma_start(
            out=T.Qt[:, a, :], in_=Qd[b, a * 128 : (a + 1) * 128, :]
        )


def _phase_A_casts(nc, P, K, T, Od, b):
    """bf16 casts (split ACT/DVE) + o1 passthrough stores."""
    Qt = T.Qt
    T.Qtb = P.mid.tile([128, KD, LQ], BF, tag="qtb", bufs=2, name=f"qtb{b}")
    T.Qmod = P.mid.tile([128, KD, LQ], BF, tag="qmod", bufs=2, name=f"qmod{b}")
    T.CtR = P.mid.tile([128, KD, LC], BF, tag="cbig", bufs=2, name=f"ctr{b}")
    for k in range(KD):
        # CtR casts split across ACT and DVE so the first S matmul
        # group isn't serialized behind one engine
        cst = T.cst[k]
        if k % 2 == 0:
            nc.scalar.activation(T.CtR[:, k, :], cst, COPY)
        else:
            nc.vector.tensor_copy(T.CtR[:, k, :], cst)
    for k in range(KD):
        # wsb cols: 0-3 w1, 4-7 w2, 8-11 w3
        nc.vector.tensor_scalar(
            out=T.Qmod[:, k, :], in0=Qt[:, k, :],
            scalar1=K.wsb[:, 8 + k : 9 + k], scalar2=K.wsb[:, k : k + 1],
            op0=MUL, op1=ADD,
        )
        nc.vector.tensor_copy(T.Qtb[:, k, :], Qt[:, k, :])
    for k in range(KD):
        # out rows 0..D-1 are exactly C[b]; configs emitted after the
        # casts so they don't delay cast dispatch on the sequencer
        nc.scalar.dma_start(out=Od[b, k * 128 : (k + 1) * 128, :], in_=T.cst[k])


def _phase_A_pe(nc, P, K, T, b):
    """Q^T via PE transposes + the c2 row matmul."""
    psum = P.psum
    # Qbt[:, kj, :] = Q^T j-block via PE transposes
    for kj in range(KJ):
        qps = psum.tile([128, D], BF, tag="ps", name=f"qps{b}_{kj}")
        for kd in range(KD):
            nc.tensor.transpose(
                qps[:, kd * 128 : (kd + 1) * 128],
                T.Qtb[:, kd, kj * 128 : (kj + 1) * 128],
                K.identB,
            )
        nc.vector.tensor_copy(T.Qbt[:, kj, :], qps)
    # c2[j] = Q^T w2 (PSUM row) -> bf16 row, applied per chunk as a
    # rank-1 (ones x c2) first term of the S accumulation group
    c2ps = psum.tile([1, LQ], F32, tag="ps", name=f"c2ps{b}")
    for kd in range(KD):
        nc.tensor.matmul(
            c2ps, K.wsbB[:, 4 + kd : 5 + kd], T.Qtb[:, kd, :],
            start=(kd == 0), stop=(kd == KD - 1),
        )
    T.c2row = P.mid.tile([1, LQ], BF, tag="c2row", bufs=2, name=f"c2r{b}")
    nc.vector.tensor_copy(T.c2row, c2ps)


def _chunk(nc, P, K, T, b, ki):
    """One 128-row i-chunk: S matmul (rank-1 c2 + contraction) -> one ACT
    exp with fused R1 row-sum -> e2cn (=S1 rows) -> DMA-transpose into
    E1T. Also PE-transposes this chunk's C^T block for the trailing
    T2/ssum consumer. Returns (e2c, cbt)."""
    psum = P.psum
    isl = slice(ki * 128, (ki + 1) * 128)
    cbt_ps = psum.tile([128, D], BF, tag="ps", name=f"cps{b}_{ki}")
    for kd in range(KD):
        nc.tensor.transpose(
            cbt_ps[:, kd * 128 : (kd + 1) * 128], T.CtR[:, kd, isl], K.identB
        )
    cbt = P.stream.tile([128, D], BF, tag="cbt", bufs=4, name=f"cbt{b}_{ki}")
    nc.vector.tensor_copy(cbt, cbt_ps)
    e2ps = psum.tile([128, LQ], F32, tag="ps", name=f"e2ps{b}_{ki}")
    nc.tensor.matmul(e2ps, K.ones_row, T.c2row, start=True, stop=False)
    for kd in range(KD):
        nc.tensor.matmul(
            e2ps, T.CtR[:, kd, isl], T.Qmod[:, kd, :],
            start=False, stop=(kd == KD - 1),
        )
    e2c = P.stream.tile([128, LQ], BF, tag="e2c", bufs=6, name=f"e2c{b}_{ki}")
    r1 = P.stream.tile([128, 1], F32, tag="r1", bufs=10, name=f"r1_{b}_{ki}")
    nc.scalar.activation(e2c, e2ps, EXP, accum_out=r1)
    r1rec = P.stream.tile([128, 1], F32, tag="r1rec", bufs=10, name=f"r1r{b}_{ki}")
    nc.vector.reciprocal(r1rec, r1)
    e2cn = P.stream.tile([128, LQ], BF, tag="e2cn", bufs=8, name=f"e2cn{b}_{ki}")
    nc.vector.tensor_scalar(
        out=e2cn, in0=e2c, scalar1=r1rec, scalar2=None, op0=MUL
    )
    # E1T[:, kj, ki*128:(ki+1)*128] = e2cn^T = S1^T columns for this chunk
    nc.sync.dma_start(out=T.E1T[:, :, isl], in_=e2cn, transpose=True)
    return e2c, cbt


def _consume(nc, P, K, T, b, ki, pair):
    """T2' accumulation + ssum (R2') for chunk ki."""
    e2c, cbt = pair
    nc.tensor.matmul(
        T.ssps, K.ones_col, e2c, start=(ki == 0), stop=(ki == MI - 1)
    )
    for mj in range(KJ):
        nc.tensor.matmul(
            T.t2ps[mj], e2c[:, mj * 128 : (mj + 1) * 128], cbt,
            start=(ki == 0), stop=(ki == MI - 1),
        )


def _a_block(nc, P, K, T, Od, b, ni, defer=False):
    """A-path outputs for one 512-col i-block: o2 = A^T, o3 = C*A^T.
    With defer=True the store configs are queued on T.pending and
    trickled out between subsequent chunks so DGE-config bursts never
    sit between ACT dispatches on the scalar sequencer."""
    psum = P.psum
    nsl = slice(ni * 512, (ni + 1) * 512)
    o2 = P.ost.tile([128, KD, 512], F32, tag="o2", bufs=2, name=f"o2_{b}_{ni}")
    o3 = P.ost.tile([128, KD, 512], F32, tag="o3", bufs=2, name=f"o3_{b}_{ni}")
    for md in range(KD):
        aps = psum.tile([128, 512], F32, tag="ps", name=f"aps{b}_{md}_{ni}")
        for kj in range(KJ):
            nc.tensor.matmul(
                aps, T.Qbt[:, kj, md * 128 : (md + 1) * 128], T.E1T[:, kj, nsl],
                start=(kj == 0), stop=(kj == KJ - 1),
            )
        nc.scalar.activation(o2[:, md, :], aps, COPY)
        nc.vector.tensor_mul(o3[:, md, :], o2[:, md, :], T.CtR[:, md, nsl])
    # one merged 3D-AP store per block: rows D+md*128+p, cols nsl
    nc.scalar.dma_start(
        out=Od[b, D : 2 * D, nsl].rearrange("(m p) c -> p m c", p=128), in_=o2
    )
    nc.gpsimd.dma_start(
        out=Od[b, 2 * D : 3 * D, nsl].rearrange("(m p) c -> p m c", p=128), in_=o3
    )


def _rec2_block(nc, P, K, T, b, stage):
    """R2' row -> per-partition reciprocal -> T2s = S2^T@Cb in bf16.
    stage 0 emits the PE transposes; stage 1 the DVE scale (so PE work
    can be slotted in between)."""
    psum = P.psum
    if stage == 0:
        T.ssrow = P.stream.tile([1, LQ], F32, tag="ssrow", name=f"ssr{b}")
        nc.vector.tensor_copy(T.ssrow, T.ssps)
        T.rc2ps = psum.tile([128, KJ], F32, tag="ps", name=f"rc2ps{b}")
        for jm in range(KJ):
            nc.tensor.transpose(
                T.rc2ps[:, jm : jm + 1],
                T.ssrow[:, jm * 128 : (jm + 1) * 128],
                K.ident[:1, :1],
            )
    else:
        rec2 = P.stream.tile([128, KJ], F32, tag="rec2", name=f"rc2{b}")
        nc.vector.reciprocal(rec2, T.rc2ps)
        for mj in range(KJ):
            nc.vector.tensor_scalar(
                out=T.T2s[:, mj, :], in0=T.t2ps[mj],
                scalar1=rec2[:, mj : mj + 1], scalar2=None, op0=MUL,
            )


def _bt_block(nc, P, K, T, Od, b, ni):
    """Bt-path outputs for one 512-col i-block: o4 = C*Bt^T."""
    psum = P.psum
    nsl = slice(ni * 512, (ni + 1) * 512)
    o4 = P.ost.tile([128, KD, 512], F32, tag="o4", bufs=2, name=f"o4_{b}_{ni}")
    for md in range(KD):
        bps = psum.tile([128, 512], F32, tag="ps", name=f"bps{b}_{md}_{ni}")
        for kj in range(KJ):
            nc.tensor.matmul(
                bps, T.T2s[:, kj, md * 128 : (md + 1) * 128], T.E1T[:, kj, nsl],
                start=(kj == 0), stop=(kj == KJ - 1),
            )
        nc.vector.tensor_mul(o4[:, md, :], bps, T.CtR[:, md, nsl])
    oeng = nc.gpsimd if ni % 2 == 0 else nc.scalar
    oeng.dma_start(
        out=Od[b, 3 * D : 4 * D, nsl].rearrange("(m p) c -> p m c", p=128), in_=o4
    )


def _alloc_big(P, T, b):
    T.E1T = P.big.tile([128, KJ, LC], BF, tag="e1t", bufs=2, name=f"e1t{b}")
    T.Qbt = P.big.tile([128, KJ, D], BF, tag="qbt", bufs=2, name=f"qbt{b}")
    T.T2s = P.big.tile([128, KJ, D], BF, tag="t2s", bufs=2, name=f"t2s{b}")


def _example(nc, P, K, T, Cd, Qd, Od, b, Tnext, phase_a_done):
    """Emit one example. The A-path blocks trail the chunk loop (their
    E1T columns arrive via DMA-transpose a few chunks earlier); the Bt
    path runs after the all-i barrier with the rec2 chain hidden under
    matmul blocks. The next example's input loads are prefetched early
    in the chunk loop and its casts/Q-transposes are emitted inside this
    example's tail so they never queue behind the tail's output work."""
    psum = P.psum
    _alloc_big(P, T, b)
    _phase_A_casts(nc, P, K, T, Od, b)
    _phase_A_pe(nc, P, K, T, b)
    T.t2ps = [
        psum.tile([128, D], F32, tag="ps", name=f"t2ps{b}_{m}") for m in range(KJ)
    ]
    T.ssps = psum.tile([1, LQ], F32, tag="ps", name=f"ssps{b}")
    e2cs = {}
    for ki in range(MI):
        e2cs[ki] = _chunk(nc, P, K, T, b, ki)
        if ki >= 2:
            _consume(nc, P, K, T, b, ki - 2, e2cs.pop(ki - 2))
        if ki == 2 and Tnext is not None:
            _loads(nc, P, K, Tnext, Cd, Qd, b + 1)
        # A-blocks trail their E1T columns by 6+ chunks to cover the
        # DMA-transpose latency.
        if ki == 7:
            _a_block(nc, P, K, T, Od, b, 0)
        if ki == 11:
            _a_block(nc, P, K, T, Od, b, 1)
    _consume(nc, P, K, T, b, MI - 2, e2cs.pop(MI - 2))
    _consume(nc, P, K, T, b, MI - 1, e2cs.pop(MI - 1))
    _rec2_block(nc, P, K, T, b, 0)  # ssrow copy + PE transposes
    _a_block(nc, P, K, T, Od, b, 2)
    _rec2_block(nc, P, K, T, b, 1)  # reciprocal + T2s scale (DVE)
    _a_block(nc, P, K, T, Od, b, 3)
    for ni in range(NI):
        _bt_block(nc, P, K, T, Od, b, ni)


def build(bl=BL, num_devices=NCORES, enable_asserts=False):
    from contextlib import ExitStack

    nc = bacc.Bacc(
        "TRN2",
        target_bir_lowering=False,
        debug=False,
        enable_asserts=enable_asserts,
        num_devices=num_devices,
    )
    Cd = nc.dram_tensor("C", (bl, D, LC), F32, kind="ExternalInput").ap()
    Qd = nc.dram_tensor("Q", (bl, D, LQ), F32, kind="ExternalInput").ap()
    wd = nc.dram_tensor("w", (3 * D,), F32, kind="ExternalInput").ap()
    Od = nc.dram_tensor("out", (bl, 4 * D, LC), F32, kind="ExternalOutput").ap()

    with tile.TileContext(nc) as tc, ExitStack() as ctx:
        P = _pools(tc, ctx)
        K = Ctx()
        tiles = {b: Ctx() for b in range(bl)}
        _loads(nc, P, K, tiles[0], Cd, Qd, 0, first=True)
        K.ident = P.const.tile([128, 128], F32, name="ident")
        make_identity(nc, K.ident)
        K.identB = P.const.tile([128, 128], BF, name="identB")
        nc.vector.tensor_copy(K.identB, K.ident)
        K.ones_col = P.const.tile([128, 1], BF, name="oc")
        nc.vector.memset(K.ones_col, 1.0)
        K.ones_row = P.const.tile([1, 128], BF, name="orr")
        nc.vector.memset(K.ones_row, 1.0)
        K.wsb = P.const.tile([128, 12], F32, name="wsb")
        nc.sync.dma_start(out=K.wsb, in_=wd.rearrange("(c p) -> p c", p=128))
        K.wsbB = P.const.tile([128, 12], BF, name="wsbB")
        nc.vector.tensor_copy(K.wsbB, K.wsb)

        for b in range(bl):
            Tnext = tiles[b + 1] if b + 1 < bl else None
            _example(nc, P, K, tiles[b], Cd, Qd, Od, b, Tnext, b > 0)
    nc.compile()
    return nc


_NC = None


def kernel(C, Q, cmask, qmask, w):
    global _NC
    C = np.ascontiguousarray(np.asarray(C, dtype=np.float32))
    Q = np.ascontiguousarray(np.asarray(Q, dtype=np.float32))
    w = np.ascontiguousarray(np.asarray(w, dtype=np.float32))
    # masks are all-ones per the problem spec; softmax masking is a no-op
    if _NC is None:
        _NC = build()
    in_maps = [
        {
            "C": np.ascontiguousarray(C[i * BL : (i + 1) * BL]),
            "Q": np.ascontiguousarray(Q[i * BL : (i + 1) * BL]),
            "w": w,
        }
        for i in range(NCORES)
    ]
    res = run_bass_kernel_spmd(_NC, in_maps, core_ids=list(range(NCORES)))
    return np.concatenate([res.results[i]["out"] for i in range(NCORES)], axis=0)
